# revision 42
# baseline (speedup 1.0000x reference)
"""MoE LoRA linear layer kernel for Trainium2, data-parallel over 8 NeuronCores.

Math (per token n):
    down = h @ down_w.T                      [N, 64]
    mask[n, r] = val[n, k] if idx[n, k] == r else 0   (indices distinct per row)
    out = (down * mask) @ up_w.T             [N, 4096]

Sharding: tokens split 8 ways (2048/core); LoRA weights replicated.

Key layout decisions (all host-side prep; HW does both matmuls + masking):
  - h is transposed + bf16-cast on the host and packed so each token tile
    streams as contiguous 2 MiB DMAs.  This removes the 512 on-device PE
    transposes + 8.4M elements of PSUM->SBUF copies a natural-layout path
    needs.
  - The top-k scatter (idx/val -> dense maskT) is host-packed like the
    baseline's idx/val chunk repack; the value multiply happens on device
    (DVE) against the down-projection PSUM result.
  - Output is stored bf16 (2e-2 rel-err budget; bf16 ~6e-3) and upcast on
    the host, halving store traffic.

PE array packing (tile_position): rank=64 only half-fills the 128x128 PE.
Tokens are split into lo/hi 256-token halves per 512-token tile:
  - down-proj: per ki, TWO concurrent matmuls at col-groups (0,0)/(0,64)
    write psum partitions 0-63 (lo tokens) and 64-127 (hi tokens).
  - mask multiply: one [128, 256] DVE op (all partitions busy).
  - up-proj: chunks from the lo/hi halves run concurrently at row-groups
    (0,0)/(64,0); upT is host-duplicated onto both partition halves.
This roughly halves PE busy time vs the unpacked schedule.

Queue assignment (measured, see git of trial runs):
  - loads prefetched up front on the sync HWDGE ring (no waits -> streams
    continuously at ~300+ GB/s);
  - early stores on the SWDGE (gpsimd) ring, late stores alternate onto
    sync once the loads drain; the scalar engine never issues DMA (its
    FIFO is occupied by PSUM copies and would head-of-line-block).

HBM traffic per core: 16 MiB in + 16 MiB out + ~2 MiB weights, against a
measured ~430 GB/s aggregate DMA envelope.
"""

import sys

for p in ("/opt/trn_rl_repo", "/opt/pypackages"):
    if p not in sys.path:
        sys.path.insert(0, p)

import numpy as np

N, D_IN, D_OUT, RANK, TOPK = 16384, 4096, 4096, 64, 8
NCORES = 8
NT = N // NCORES          # tokens per core = 2048
P = 128                   # partitions
TT = 512                  # token tile
HT = TT // 2              # lo/hi token half-tile (256)
NKC = D_IN // P           # 32 contraction chunks for down proj
NTILES = NT // TT         # 4 token tiles per core
NJ = TT // P              # 4 x 128-token chunks per tile
OT = 512                  # output col tile (one PSUM bank)
NOT = D_OUT // OT         # 8 output col tiles
HK = NKC // 2             # ki chunks per ht half (2 MiB loads)

_CACHE = {}


def _build_program():
    import concourse.bacc as bacc
    import concourse.mybir as mybir
    from concourse import tile

    f32 = mybir.dt.float32
    bf16 = mybir.dt.bfloat16
    # Bacc (not plain Bass): its finalize() runs move_matmul_waits_to_-
    # ldweights + generate_event_semaphores, which split semaphore waits to
    # satisfy the TRN2 one-wait-per-instruction constraint.
    nc = bacc.Bacc()

    ht = nc.declare_dram_parameter("ht", [NTILES * P, NKC * TT], bf16, isOutput=False)
    dwt = nc.declare_dram_parameter("dwt", [P, NKC * RANK], bf16, isOutput=False)
    upw = nc.declare_dram_parameter("upw", [P, D_OUT], bf16, isOutput=False)
    maskt = nc.declare_dram_parameter("maskt", [P, NT // 2], f32, isOutput=False)
    out = nc.declare_dram_parameter("out", [NT, D_OUT], bf16, isOutput=True)

    with tile.TileContext(nc) as tc:
        with (
            tc.tile_pool(name="const", bufs=1) as const,
            tc.tile_pool(name="ht", bufs=7) as ht_pool,
            tc.tile_pool(name="resT", bufs=2) as resT_pool,
            tc.tile_pool(name="outsb", bufs=4) as out_pool,
            tc.tile_pool(name="psum_dn", bufs=2, space="PSUM") as psum_dn_pool,
            tc.tile_pool(name="psum_up", bufs=3, space="PSUM") as psum_up_pool,
        ):
            dwt_sb = const.tile([P, NKC * RANK], bf16)
            upT_sb = const.tile([P, D_OUT], bf16)
            maskT_sb = const.tile([P, NT // 2], f32)
            nc.sync.dma_start(out=dwt_sb[:], in_=dwt[:, :])
            # upw/maskt ride the store (SWDGE) queue, which is empty until
            # the first up-proj finishes.
            nc.gpsimd.dma_start(out=upT_sb[:], in_=upw[:, :])
            nc.gpsimd.dma_start(out=maskT_sb[:], in_=maskt[:, :])

            copy_engines = [nc.vector.tensor_copy, nc.scalar.copy]

            # Prefetch the whole ht stream up front: load dma_starts carry
            # no waits, so the sync ring streams continuously at full rate,
            # and late stores queued on sync (below) drain right after.
            ht_halves_all = []
            for tt in range(NTILES):
                for hh in range(2):
                    ht_sb = ht_pool.tile([P, HK * TT], bf16)
                    nc.sync.dma_start(
                        out=ht_sb[:],
                        in_=ht[tt * P:(tt + 1) * P,
                               hh * HK * TT:(hh + 1) * HK * TT],
                    )
                    ht_halves_all.append(ht_sb)

            for tt in range(NTILES):
                ht_halves = ht_halves_all[tt * 2:tt * 2 + 2]

                # psum_dn partitions 0-63 accumulate lo tokens (u<256),
                # partitions 64-127 hi tokens; full-bank tile so the two
                # pool buffers never share a PSUM bank.
                psum_dn = psum_dn_pool.tile([P, TT], f32)
                if tt == 0:
                    # PE warm-up: HAM starts kernels throttled to 1.2 GHz
                    # and needs ~3.4us of sustained matmul activity to
                    # unthrottle.  Bridge the gap between dwt landing and
                    # the first ht half with throwaway matmuls over dwt (a
                    # closed accumulation group; the real groups' start=True
                    # clears has_written, so the garbage never leaks).
                    for wi in range(40):
                        nc.tensor.matmul(
                            psum_dn[:RANK, :],
                            lhsT=dwt_sb[:, :RANK],
                            rhs=dwt_sb[:, :TT],
                            start=(wi == 0),
                            stop=(wi == 39),
                        )
                for ki in range(NKC):
                    src = ht_halves[ki // HK]
                    base = (ki % HK) * TT
                    # two concurrent matmuls on distinct PE col-groups
                    nc.tensor.matmul(
                        psum_dn[0:RANK, 0:HT],
                        lhsT=dwt_sb[:, ki * RANK:(ki + 1) * RANK],
                        rhs=src[:, base:base + HT],
                        start=(ki == 0),
                        stop=(ki == NKC - 1),
                        tile_position=(0, 0),
                    )
                    nc.tensor.matmul(
                        psum_dn[RANK:P, 0:HT],
                        lhsT=dwt_sb[:, ki * RANK:(ki + 1) * RANK],
                        rhs=src[:, base + HT:base + TT],
                        start=(ki == 0),
                        stop=(ki == NKC - 1),
                        tile_position=(0, RANK),
                    )

                resT = resT_pool.tile([P, HT], bf16)
                nc.vector.tensor_mul(
                    resT[:],
                    maskT_sb[:, tt * HT:(tt + 1) * HT],
                    psum_dn[:, 0:HT],
                )

                # up-proj: chunk pairs (lo-half chunk j, hi-half chunk j+2)
                # run concurrently on PE row-groups (0,0)/(64,0).
                for jp in range(NJ // 2):
                    out_lo = out_pool.tile([P, D_OUT], bf16)
                    out_hi = out_pool.tile([P, D_OUT], bf16)
                    for o in range(NOT):
                        psum_lo = psum_up_pool.tile([P, OT], f32)
                        nc.tensor.matmul(
                            psum_lo[:],
                            lhsT=resT[0:RANK, jp * P:(jp + 1) * P],
                            rhs=upT_sb[0:RANK, o * OT:(o + 1) * OT],
                            start=True,
                            stop=True,
                            tile_position=(0, 0),
                        )
                        psum_hi = psum_up_pool.tile([P, OT], f32)
                        nc.tensor.matmul(
                            psum_hi[:],
                            lhsT=resT[RANK:P, jp * P:(jp + 1) * P],
                            rhs=upT_sb[RANK:P, o * OT:(o + 1) * OT],
                            start=True,
                            stop=True,
                            tile_position=(RANK, 0),
                        )
                        # one engine per half so both copies run in
                        # parallel on different PSUM banks
                        cp = copy_engines[o % 2]
                        cp2 = copy_engines[(o + 1) % 2]
                        cp(out=out_lo[:, o * OT:(o + 1) * OT], in_=psum_lo[:])
                        cp2(out=out_hi[:, o * OT:(o + 1) * OT], in_=psum_hi[:])
                    # lo chunk jp -> tokens tt*512 + jp*128
                    # hi chunk jp -> tokens tt*512 + 256 + jp*128
                    for half, osb in ((0, out_lo), (1, out_hi)):
                        row = tt * TT + half * HT + jp * P
                        # Time-phased store queues: early stores must drain
                        # promptly (SWDGE; sync is mid-load-stream); late
                        # tiles alternate onto sync, whose HWDGE ring frees
                        # up as the prefetched loads end, so the two store
                        # streams finish together.
                        if tt < 2 or (jp * 2 + half) % 2 == 0:
                            nc.gpsimd.dma_start(
                                out=out[row:row + P, :], in_=osb[:]
                            )
                        else:
                            nc.sync.dma_start(
                                out=out[row:row + P, :], in_=osb[:]
                            )

    nc.finalize()
    return nc


def _get_program():
    if "nc" not in _CACHE:
        _CACHE["nc"] = _build_program()
    return _CACHE["nc"]


def prepare_in_maps(hidden_states, down_w, up_w, top_k_values, top_k_indices):
    import ml_dtypes

    bf16 = ml_dtypes.bfloat16

    h = np.asarray(hidden_states, dtype=np.float32)
    dw = np.asarray(down_w, dtype=np.float32)
    uw = np.asarray(up_w, dtype=np.float32)
    vals = np.asarray(top_k_values, dtype=np.float32)
    idx = np.asarray(top_k_indices).astype(np.int64)

    # dwT[p, ki*64 + r] = dw[r, ki*128 + p]
    dwT = np.ascontiguousarray(
        dw.reshape(RANK, NKC, P).transpose(2, 1, 0).reshape(P, NKC * RANK)
    ).astype(bf16)
    # upT duplicated onto both partition halves for the row-group-paired
    # up-proj: upT2[r, x] = upT2[64 + r, x] = up_w[x, r]
    upT = np.ascontiguousarray(uw.T)  # [64, 4096]
    upT2 = np.concatenate([upT, upT], axis=0).astype(bf16)  # [128, 4096]

    # dense scatter of top-k values: mask[n, r] = val[n, k] where idx[n,k]==r
    mask = np.zeros((N, RANK), dtype=np.float32)
    rows = np.arange(N)[:, None]
    mask[rows, idx] = vals

    in_maps = []
    for c in range(NCORES):
        s = slice(c * NT, (c + 1) * NT)
        # ht[tt*128 + p, ki*512 + u] = h[c*NT + tt*512 + u, ki*128 + p]
        ht = (
            h[s]
            .reshape(NTILES, TT, NKC, P)
            .transpose(0, 3, 2, 1)
            .reshape(NTILES * P, NKC * TT)
            .astype(bf16)
        )
        # maskT2[r + 64*half, tt*256 + u] = mask[c*NT + tt*512 + half*256 + u, r]
        m = mask[s].reshape(NTILES, 2, HT, RANK)  # [tt, half, u, r]
        maskT2 = np.ascontiguousarray(
            m.transpose(1, 3, 0, 2).reshape(2 * RANK, NTILES * HT)
        )
        in_maps.append(
            {
                "ht": np.ascontiguousarray(ht),
                "dwt": dwT,
                "upw": upT2,
                "maskt": maskT2,
            }
        )
    return in_maps


def kernel(hidden_states, down_w, up_w, top_k_values, top_k_indices, **_kw):
    from concourse.bass_utils import run_bass_kernel_spmd

    nc = _get_program()
    in_maps = prepare_in_maps(
        hidden_states, down_w, up_w, top_k_values, top_k_indices
    )
    res = run_bass_kernel_spmd(nc, in_maps, core_ids=list(range(NCORES)))
    return np.concatenate(
        [_unpack_out(r["out"]) for r in res.results], axis=0
    )


def _unpack_out(o):
    return np.asarray(o, dtype=np.float32)
